# revision 6
# baseline (speedup 1.0000x reference)
"""ChannelAttention kernel for 8 Trainium2 NeuronCores (Bass/Tile, SPMD).

Problem (hardcoded shapes):
  x: (4, 8, 32, 32, 512) fp32, Wqkv: (1536, 512), Wproj: (512, 512), bproj: (512,)
  reference: per (b, head): attn = softmax((k*scale)^T @ v) over head_dim;
             out = (attn @ q^T)^T; y = out @ Wproj^T + bproj

Sharding: tokens (B*D*H*W = 32768) split across 8 cores -> 4096 tokens/core.
Cores (2j, 2j+1) hold the two halves of batch j. The only cross-core data
is the per-head 64x64 k^T v partial sums: one pairwise AllReduce of
8*64*64 fp32 = 128KB. Everything else (qkv, softmax, out, proj) is local.

Matmuls run in float32r (fp32 storage, reduced-precision PE multiply) at
full PE rate for N>=256; accumulation is fp32 in PSUM.
"""

import numpy as np
from contextlib import ExitStack

import concourse.bass as bass
import concourse.mybir as mybir
import concourse.tile as tile
from concourse import bacc
from concourse.bass_utils import run_bass_kernel_spmd
from concourse.masks import make_identity

# ---- problem constants -----------------------------------------------------
B, D, H, W, C = 4, 8, 32, 32, 512
NUM_HEADS = 8
HEAD_DIM = C // NUM_HEADS          # 64
SCALE = HEAD_DIM ** -0.5
N_TOK = B * D * H * W              # 32768 tokens total
N_CORES = 8
N_LOC = N_TOK // N_CORES           # 4096 tokens per core
CHUNK = 512                        # token chunk (matmul free dim)
N_CHUNKS = N_LOC // CHUNK          # 8
TT = 128                           # token tile (partition dim)
T_PER_CHUNK = CHUNK // TT          # 4
N_CI = C // 128                    # 4 ci-tiles of 128 channels
N_PAIRS = NUM_HEADS // 2           # 4 head-pairs (128 channels each)

f32 = mybir.dt.float32
f32r = mybir.dt.float32r

REPLICA_GROUPS = [[0, 1], [2, 3], [4, 5], [6, 7]]

_NC_CACHE = None


def build_nc():
    """Build + compile the SPMD Bass program (identical on all 8 cores)."""
    nc = bacc.Bacc(num_devices=N_CORES)

    # Per-core inputs (fp32 bytes, tagged f32r so the PE consumes them directly)
    xT = nc.declare_dram_parameter("xT", [C, N_LOC], f32r, isOutput=False)
    wq = nc.declare_dram_parameter("wq", [C, C], f32r, isOutput=False)       # Wqkv[0:512].T
    wkv = nc.declare_dram_parameter("wkv", [C, 2 * C], f32r, isOutput=False)  # [k*scale | v] cols
    wp = nc.declare_dram_parameter("wp", [C, C], f32r, isOutput=False)       # Wproj.T
    bp = nc.declare_dram_parameter("bp", [1, C], f32r, isOutput=False)       # bproj row
    y = nc.declare_dram_parameter("y", [N_LOC, C], f32, isOutput=True)

    # DRAM views for tiled DMA: partition = 128 channels, blocks of ci-tiles
    xT_v = xT.rearrange("(a p) n -> p a n", p=128)        # [128, 4, N_LOC]
    wq_v = wq.rearrange("(a p) f -> p a f", p=128)        # [128, 4, 512]
    wkv_v = wkv.rearrange("(a p) f -> p a f", p=128)      # [128, 4, 1024]
    wp_v = wp.rearrange("(a p) f -> p a f", p=128)        # [128, 4, 512]

    with tile.TileContext(nc) as tc, ExitStack() as ctx:
        const = ctx.enter_context(tc.tile_pool(name="const", bufs=1))
        persist = ctx.enter_context(tc.tile_pool(name="persist", bufs=1))
        sb = ctx.enter_context(tc.tile_pool(name="sb", bufs=2))
        kvp = ctx.enter_context(tc.tile_pool(name="kvp", bufs=4))
        dram = ctx.enter_context(tc.tile_pool(name="dram", bufs=1, space="DRAM"))

        # ---- constants / weights in SBUF ----
        wq_sb = const.tile([128, N_CI, C], f32r)
        nc.sync.dma_start(wq_sb[:], wq_v[:])
        wkv_sb = const.tile([128, N_CI, 2 * C], f32r)
        nc.sync.dma_start(wkv_sb[:], wkv_v[:])
        wp_sb = const.tile([128, N_CI, C], f32r)
        nc.sync.dma_start(wp_sb[:], wp_v[:])
        bp_sb = const.tile([1, C], f32r)
        nc.sync.dma_start(bp_sb[:], bp[:])
        ones_f32 = const.tile([1, 128], f32)
        nc.vector.memset(ones_f32[:], 1.0)
        ones_sb = const.tile([1, 128], f32r)
        nc.vector.tensor_copy(ones_sb[:], ones_f32[:])
        zrow_f32 = const.tile([1, 512], f32)
        nc.vector.memset(zrow_f32[:], 0.0)
        zrow_sb = const.tile([1, 512], f32r)
        nc.vector.tensor_copy(zrow_sb[:], zrow_f32[:])
        ident = const.tile([128, 128], f32)
        make_identity(nc, ident[:])

        # qT for all chunks, consumed after the collective: [128, pair, chunk, 512]
        qT_all = persist.tile([128, N_PAIRS, N_CHUNKS, CHUNK], f32r)

        # ================= phase 1: qkv + attn partial accumulation =========
        with (
            tc.tile_pool(name="ps_q", bufs=2, space="PSUM") as ps_q,
            tc.tile_pool(name="ps_kv", bufs=2, space="PSUM") as ps_kv,
            tc.tile_pool(name="ps_at", bufs=1, space="PSUM") as ps_at,
        ):
            # attn accumulator: [128 (d of head-pair), pair, 256 (e of head-quad)]
            # Seed each bank once with a K=1 zero matmul: start=True clears
            # has_written BANK-WIDE, so per-pair groups sharing a bank must
            # not each issue their own start.
            attn_ps = ps_at.tile([128, N_PAIRS, 256], f32)
            for bank in range(2):
                nc.tensor.matmul(
                    attn_ps[:, 2 * bank:2 * bank + 2, :].rearrange("p a e -> p (a e)"),
                    ones_sb[:], zrow_sb[:],
                    start=True, stop=False, skip_group_check=True,
                )

            for c in range(N_CHUNKS):
                xt = sb.tile([128, N_CI, CHUNK], f32r, tag="xt")
                nc.sync.dma_start(xt[:], xT_v[:, :, c * CHUNK:(c + 1) * CHUNK])

                # k|v for this chunk: per token-tile [128 tok, 1024 (k|v)]
                kv_tiles = []
                for s in range(T_PER_CHUNK):
                    kv_ps = ps_kv.tile([128, 2 * C], f32, tag="kv")
                    for h in range(2):  # k half, v half
                        for k in range(N_CI):
                            nc.tensor.matmul(
                                kv_ps[:, h * C:(h + 1) * C],
                                xt[:, k, s * TT:(s + 1) * TT],
                                wkv_sb[:, k, h * C:(h + 1) * C],
                                start=(k == 0), stop=(k == N_CI - 1),
                            )
                    kv_sb = kvp.tile([128, 2 * C], f32r, tag="kvsb")
                    nc.vector.tensor_copy(kv_sb[:], kv_ps[:])
                    kv_tiles.append(kv_sb)

                # qT for this chunk: [f-pair 128, tok 512] per pair
                for p in range(N_PAIRS):
                    q_ps = ps_q.tile([128, CHUNK], f32, tag="q")
                    for k in range(N_CI):
                        nc.tensor.matmul(
                            q_ps[:],
                            wq_sb[:, k, p * 128:(p + 1) * 128],
                            xt[:, k, :],
                            start=(k == 0), stop=(k == N_CI - 1),
                        )
                    nc.scalar.copy(qT_all[:, p, c, :], q_ps[:])

                # attn partial accumulation: pair p vs its head-quad of v
                for s in range(T_PER_CHUNK):
                    kv_sb = kv_tiles[s]
                    for p in range(N_PAIRS):
                        q4 = p // 2  # head-quad index
                        nc.tensor.matmul(
                            attn_ps[:, p, :],
                            kv_sb[:, p * 128:(p + 1) * 128],               # k pair
                            kv_sb[:, C + q4 * 256:C + (q4 + 1) * 256],     # v quad
                            start=False,
                            stop=(c == N_CHUNKS - 1 and s == T_PER_CHUNK - 1),
                            skip_group_check=True,
                        )

            # ---- pack useful 64x64 blocks: [64 (d), head, 64 (e)] ----
            cc_sb = sb.tile([64, NUM_HEADS, 64], f32, tag="ccsb")
            for h in range(NUM_HEADS):
                p = h // 2
                row0 = (h % 2) * 64
                col0 = (p % 2) * 128 + (h % 2) * 64
                nc.vector.tensor_copy(
                    cc_sb[:, h, :],
                    attn_ps[row0:row0 + 64, p, col0:col0 + 64],
                )

        # ================= collective: pairwise AllReduce of partials =======
        cc_in = dram.tile([64, NUM_HEADS * 64], f32)
        cc_out = dram.tile([64, NUM_HEADS * 64], f32)
        nc.sync.dma_start(cc_in[:], cc_sb.rearrange("p h e -> p (h e)"))
        nc.gpsimd.collective_compute(
            "AllReduce",
            mybir.AluOpType.add,
            replica_groups=REPLICA_GROUPS,
            ins=[cc_in.opt()],
            outs=[cc_out.opt()],
        )
        attn_sb = sb.tile([64, NUM_HEADS, 64], f32, tag="attnsb")
        nc.sync.dma_start(attn_sb.rearrange("p h e -> p (h e)"), cc_out[:])

        # ================= softmax over e (last axis), batched over heads ===
        nmax = sb.tile([64, NUM_HEADS, 1], f32, tag="nmax")
        nc.vector.reduce_max(nmax[:], attn_sb[:], axis=mybir.AxisListType.X, negate=True)
        shifted = sb.tile([64, NUM_HEADS, 64], f32, tag="shifted")
        nc.vector.tensor_add(shifted[:], attn_sb[:], nmax.broadcast_to([64, NUM_HEADS, 64]))
        expd = sb.tile([64, NUM_HEADS, 64], f32, tag="expd")
        nc.scalar.activation(expd[:], shifted[:], mybir.ActivationFunctionType.Exp)
        ssum = sb.tile([64, NUM_HEADS, 1], f32, tag="ssum")
        nc.vector.reduce_sum(ssum[:], expd[:], axis=mybir.AxisListType.X)
        rsum = sb.tile([64, NUM_HEADS, 1], f32, tag="rsum")
        nc.vector.reciprocal(rsum[:], ssum[:])
        probs = sb.tile([64, NUM_HEADS, 64], f32, tag="probs")
        nc.vector.tensor_mul(probs[:], expd[:], rsum.broadcast_to([64, NUM_HEADS, 64]))

        # ================= transpose attn -> block-diag pair lhsT ===========
        zro = sb.tile([128, N_PAIRS, 128], f32, tag="zro")
        nc.vector.memset(zro[:], 0.0)
        atnT = persist.tile([128, N_PAIRS, 128], f32r)
        nc.vector.tensor_copy(atnT[:], zro[:])
        with tc.tile_pool(name="ps_tr", bufs=1, space="PSUM") as ps_tr:
            tr_ps = ps_tr.tile([64, NUM_HEADS, 64], f32)
            for h in range(NUM_HEADS):
                nc.tensor.transpose(tr_ps[:, h, :], probs[:, h, :], ident[0:64, 0:64])
            for h in range(NUM_HEADS):
                p = h // 2
                off = (h % 2) * 64
                nc.vector.tensor_copy(
                    atnT[off:off + 64, p, off:off + 64], tr_ps[:, h, :]
                )

        # ================= out = attnT-pair @ qT, then proj + bias ==========
        with (
            tc.tile_pool(name="ps_o", bufs=2, space="PSUM") as ps_o,
            tc.tile_pool(name="ps_y", bufs=2, space="PSUM") as ps_y,
        ):
            for c in range(N_CHUNKS):
                outT_sb = sb.tile([128, N_CI, CHUNK], f32r, tag="outT")
                for p in range(N_PAIRS):
                    o_ps = ps_o.tile([128, CHUNK], f32, tag="o")
                    nc.tensor.matmul(
                        o_ps[:], atnT[:, p, :], qT_all[:, p, c, :],
                        start=True, stop=True,
                    )
                    nc.vector.tensor_copy(outT_sb[:, p, :], o_ps[:])

                for s in range(T_PER_CHUNK):
                    y_ps = ps_y.tile([128, C], f32, tag="y")
                    # bias via K=1 ones matmul (starts the accumulation group)
                    nc.tensor.matmul(
                        y_ps[:], ones_sb[:], bp_sb[:], start=True, stop=False,
                    )
                    for k in range(N_CI):
                        nc.tensor.matmul(
                            y_ps[:],
                            outT_sb[:, k, s * TT:(s + 1) * TT],
                            wp_sb[:, k, :],
                            start=False, stop=(k == N_CI - 1),
                        )
                    y_sb = sb.tile([128, C], f32, tag="ysb")
                    nc.scalar.copy(y_sb[:], y_ps[:])
                    t0 = c * CHUNK + s * TT
                    nc.sync.dma_start(y[t0:t0 + TT, :], y_sb[:])

    nc.compile()
    return nc


def _get_nc():
    global _NC_CACHE
    if _NC_CACHE is None:
        _NC_CACHE = build_nc()
    return _NC_CACHE


def prep_inputs(x, Wqkv, Wproj, bproj):
    """Host-side shard + layout prep -> per-core input maps."""
    x = np.ascontiguousarray(np.asarray(x, dtype=np.float32))
    Wqkv = np.asarray(Wqkv, dtype=np.float32)
    Wproj = np.asarray(Wproj, dtype=np.float32)
    bproj = np.asarray(bproj, dtype=np.float32)

    xf = x.reshape(B, D * H * W, C)
    wq = np.ascontiguousarray(Wqkv[0:C].T)                       # (512, 512)
    wk = Wqkv[C:2 * C] * np.float32(SCALE)
    wv = Wqkv[2 * C:3 * C]
    wkv = np.ascontiguousarray(np.concatenate([wk, wv], axis=0).T)  # (512, 1024)
    wp = np.ascontiguousarray(Wproj.T)                           # (512, 512)
    bp = np.ascontiguousarray(bproj.reshape(1, C))

    in_maps = []
    for i in range(N_CORES):
        b = i // 2
        t0 = (i % 2) * N_LOC
        slab = xf[b, t0:t0 + N_LOC, :]                           # (4096, 512)
        xT = np.ascontiguousarray(slab.T)                        # (512, 4096)
        in_maps.append({"xT": xT, "wq": wq, "wkv": wkv, "wp": wp, "bp": bp})
    return in_maps


def gather_output(results):
    parts = [np.asarray(results[i]["y"]) for i in range(N_CORES)]
    yf = np.concatenate(parts, axis=0)                           # (32768, 512)
    return yf.reshape(B, D, H, W, C)


def kernel(x, Wqkv, Wproj, bproj, _trace=False, _tmpdir=None):
    nc = _get_nc()
    in_maps = prep_inputs(x, Wqkv, Wproj, bproj)
    res = run_bass_kernel_spmd(
        nc, in_maps, list(range(N_CORES)), trace=_trace, tmpdir=_tmpdir
    )
    out = gather_output(res.results)
    if _trace:
        kernel.last_exec_time_ns = res.exec_time_ns
        kernel.last_results = res
    return out


if __name__ == "__main__":
    rng = np.random.default_rng(0)
    x = rng.standard_normal((B, D, H, W, C), dtype=np.float32)
    Wqkv = rng.standard_normal((3 * C, C), dtype=np.float32) * C ** -0.5
    Wproj = rng.standard_normal((C, C), dtype=np.float32) * C ** -0.5
    bproj = rng.standard_normal((C,), dtype=np.float32) * 0.01
    y = kernel(x, Wqkv, Wproj, bproj)
    print("ran:", y.shape, y.dtype)
